# revision 7
# baseline (speedup 1.0000x reference)
"""Trainium2 Bass kernel for nn_DistMultMod, v8.

Decomposition (per core, BC=1024 triplets, balanced assignment):
  - comp slots (masked last-writer nodes, ~195/core): dv = sum_k w*node_emb[neigh]
    via dma_gather of neighbor rows bucketed by 32768-row block (16 ops),
    rows within an op grouped by parity tile T then padded (idx 0, w=0) to
    tight per-op static sizes (max over cores, 128-aligned), + PE matmuls
    with host-built scaled one-hot W (bf16).
  - score phase: per j-tile,
      psum_h[j] = sum_T Ah[T,j].T @ dv16[T]
      psum_r[j] = Rone[j].T @ rel_emb
      h = psum_h + g_h*old_h ; t = psum_t + g_t*old_t ; score = sum(h*t*rel)
    old head/tail rows fetched early with indirect DMAs on queue 0.

v8 vs v7:
  - tight static gather sizes (max-over-cores per (block,T), 128-aligned)
    instead of fixed 1024-row capacity: ~20% less emission + DMA.
  - balanced triplet->core assignment (writer triplets dealt round-robin)
    to keep per-core and per-op row counts tight.
  - old-row indirect DMAs issued FIRST (q0, cores 0-1) interleaved with comp
    gathers on queues 1-3 (cores 2-7): emission fully parallel.
  - dynamic_dma_scratch_size=65536: descriptor rings 4x deeper, decouples
    emission from SDMA drain.
  - idx16 split into two DMAs (first 3 scheduled ops, then rest) so gathers
    start ~2us in; Ah/At/rone/rel loads deferred behind gather issue.
  - warm-up 16-row dma_gather issued first to pull the ~6us ext-isa IRAM
    load off the critical path.
"""
import numpy as np
import ml_dtypes

BF16 = ml_dtypes.bfloat16

B = 8192
NCORES = 8
BC = B // NCORES        # 1024 triplets per core
D = 128
K = 64
N = 500000
RELS = 16
ND = 20000
NBLK = 16               # node_emb row blocks of 32768 (int16 index space)
BLK = 32768
NJ = BC // 128          # score tiles = 8
LAM = 0.7
SP = False              # single_packet for dma_gather
M_DEF = 256

_CACHE = {}


def _assign_cores(head, rel, mask):
    """Deal writer triplets round-robin across cores, fill with the rest.
    Returns perm: [NCORES, BC] original triplet indices."""
    last_of = {}
    for b in range(B):
        last_of[int(head[b])] = b
    is_writer = np.zeros(B, bool)
    for b in range(B):
        if mask[b] and last_of[int(head[b])] == b:
            is_writer[b] = True
    perm = [[] for _ in range(NCORES)]
    c = 0
    for b in np.flatnonzero(is_writer):
        perm[c % NCORES].append(int(b))
        c += 1
    c = 0
    for b in np.flatnonzero(~is_writer):
        while len(perm[c % NCORES]) >= BC:
            c += 1
        perm[c % NCORES].append(int(b))
        c += 1
    return np.array(perm, np.int64), last_of


def _prep_cores(head, rel, tailv, local_idx_map, sim_neighbors, sim_weights,
                degree_table, M):
    """Host-side prep. Returns (cores, static) or 'slots' if M too small."""
    NT = M // 128
    mask = (rel >= 2) & (rel <= 4)
    local_idx_map = np.asarray(local_idx_map)
    sim_neighbors = np.asarray(sim_neighbors)
    sim_weights = np.asarray(sim_weights)
    degree_table = np.asarray(degree_table)

    perm, last_of = _assign_cores(head, rel, mask)

    raw = []
    for c in range(NCORES):
        pc = perm[c]
        slot_of = {}
        writers = []

        def slot_for(node):
            bw = last_of.get(int(node), -1)
            if bw < 0 or not mask[bw]:
                return M - 1
            s = slot_of.get(bw, -1)
            if s < 0:
                s = len(writers)
                slot_of[bw] = s
                writers.append(bw)
            return s

        slot_h = np.empty(BC, np.int32)
        slot_t = np.empty(BC, np.int32)
        for i in range(BC):
            slot_h[i] = slot_for(head[pc[i]])
            slot_t[i] = slot_for(tailv[pc[i]])
        m = len(writers)
        if m > M - 1:
            return "slots", None

        a = np.zeros(M, np.float32)
        wl = np.array(writers, dtype=np.int64)
        if m:
            ls = local_idx_map[head[wl]]
            neigh_rows = sim_neighbors[ls].astype(np.int64)   # [m, K]
            w_rows = sim_weights[ls].astype(np.float32)       # [m, K]
            deg = degree_table[ls, rel[wl] - 2].astype(np.float32)
            a[:m] = LAM * np.exp(-LAM * deg) + 0.2
        else:
            neigh_rows = np.zeros((0, K), np.int64)
            w_rows = np.zeros((0, K), np.float32)

        srows = np.repeat(np.arange(m), K)
        rnodes = neigh_rows.reshape(-1)
        wvals = w_rows.reshape(-1)
        Tpar = (srows % NT).astype(np.int64)
        blk = rnodes >> 15
        pcol = (srows // NT).astype(np.int64)
        off = (rnodes & (BLK - 1)).astype(np.int64)
        order = np.lexsort((pcol, Tpar, blk))
        raw.append(dict(slot_h=slot_h, slot_t=slot_t, a=a, m=m, pc=pc,
                        sTs=Tpar[order], sblk=blk[order], soff=off[order],
                        sw=wvals[order], sp=pcol[order]))

    # per-(core, blk, T) counts -> static group counts (max over cores)
    cnt = np.zeros((NCORES, NBLK, NT), np.int64)
    for c in range(NCORES):
        r = raw[c]
        np.add.at(cnt[c], (r["sblk"], r["sTs"]), 1)
    G = np.maximum(np.ceil(cnt.max(axis=0) / 128).astype(np.int64), 0)  # [NBLK, NT]
    NIDX = (G.sum(axis=1) * 128).astype(np.int64)                       # [NBLK]

    # comp schedule order: blocks in plain order (round-robin q1,q2,q3 later)
    sched = list(range(NBLK))
    GT = [[int(G[op, T]) for op in sched] for T in range(NT)]
    NI = [int(NIDX[op]) for op in sched]
    static = dict(NT=NT, GT=GT, NI=NI, M=M,
                  Gmax=max(int(G[op].sum()) for op in sched) or 1)

    cores = []
    for c in range(NCORES):
        r = raw[c]
        a, slot_h, slot_t, pc = r["a"], r["slot_h"], r["slot_t"], r["pc"]
        # build per-op idx + wmat in sched order
        idx_cols = []
        wgroups = []
        for op in sched:
            for T in range(NT):
                g = int(G[op, T])
                if g == 0:
                    continue
                sel = (r["sblk"] == op) & (r["sTs"] == T)
                offs = r["soff"][sel]
                ws = r["sw"][sel]
                ps = r["sp"][sel]
                n = len(offs)
                cap = g * 128
                assert n <= cap
                idx = np.zeros(cap, np.int16)
                idx[:n] = offs
                W = np.zeros((g, 128, 128), np.float32)
                pos = np.arange(n)
                W[pos // 128, pos % 128, ps] = ws
                idx_cols.append(idx)
                wgroups.append(W.reshape(g * 128, 128))
        idx_all = (np.concatenate(idx_cols) if idx_cols
                   else np.zeros(0, np.int16))
        # idx layout: [128, total/16]; partition p, col j holds idx[j*16 + p%16]
        tc = len(idx_all) // 16
        idx16 = (np.tile(idx_all.reshape(tc, 16).T, (8, 1))
                 if tc else np.zeros((128, 1), np.int16))
        wmat = (np.concatenate(wgroups, axis=0) if wgroups
                else np.zeros((128, 128), np.float32))
        # [sumG*128, 128] -> [128, sumG*128] partition-major for lhsT slices
        wmat = np.ascontiguousarray(
            wmat.reshape(-1, 128, 128).transpose(1, 0, 2)
            .reshape(128, -1)).astype(BF16)

        i = np.arange(BC)
        j_, t_ = i // 128, i % 128
        Ah = np.zeros((128, NT * NJ * 128), np.float32)
        T_, p_ = slot_h % NT, slot_h // NT
        Ah[p_, (T_ * NJ + j_) * 128 + t_] = a[slot_h]
        At = np.zeros((128, NT * NJ * 128), np.float32)
        T_, p_ = slot_t % NT, slot_t // NT
        At[p_, (T_ * NJ + j_) * 128 + t_] = a[slot_t]
        gh = np.ascontiguousarray((1.0 - a[slot_h]).reshape(NJ, 128).T)
        gt = np.ascontiguousarray((1.0 - a[slot_t]).reshape(NJ, 128).T)
        rone = np.zeros((RELS, NJ * 128), np.float32)
        rone[rel[pc], i] = 1.0

        cores.append(dict(
            idx16=np.ascontiguousarray(idx16),
            wmat=wmat,
            Ah=np.ascontiguousarray(Ah).astype(BF16),
            At=np.ascontiguousarray(At).astype(BF16),
            gh=gh.astype(np.float32),
            gt=gt.astype(np.float32),
            rone=np.ascontiguousarray(rone),
            headi=np.ascontiguousarray(
                head[pc].reshape(NJ, 128).T).astype(np.int32),
            taili=np.ascontiguousarray(
                tailv[pc].reshape(NJ, 128).T).astype(np.int32),
        ))
    return cores, (static, perm)


def _build_nc(static):
    import concourse.bass as bass
    import concourse.bacc as bacc
    import concourse.mybir as mybir
    import concourse.tile as tile

    NT = static["NT"]
    GT = static["GT"]          # [NT][NOP] group counts
    NI = static["NI"]          # [NOP] static num_idxs per op
    Gmax = static["Gmax"]
    NOP = len(NI)
    f32 = mybir.dt.float32
    bf16 = mybir.dt.bfloat16
    i32 = mybir.dt.int32
    i16 = mybir.dt.int16
    Alu = mybir.AluOpType

    TOTC = sum(ni // 16 for ni in NI)        # idx16 columns
    TOTG = sum(sum(GT[T]) for T in range(NT))  # total weight groups
    # per-op offsets
    colo = np.cumsum([0] + [ni // 16 for ni in NI]).tolist()
    gofs = []
    acc = 0
    for op in range(NOP):
        gofs.append(acc)
        acc += sum(GT[T][op] for T in range(NT))

    nc = bacc.Bacc("TRN2", target_bir_lowering=False, debug=False,
                   num_devices=NCORES, num_swdge_queues=4,
                   dynamic_dma_scratch_size=65536)

    node_emb = nc.dram_tensor("node_emb", [N, D], f32, kind="ExternalInput")
    rel_emb = nc.dram_tensor("rel_emb", [RELS, D], f32, kind="ExternalInput")
    idx16_t = nc.dram_tensor("idx16", [128, max(TOTC, 1)], i16,
                             kind="ExternalInput")
    wmat_t = nc.dram_tensor("wmat", [128, max(TOTG, 1) * 128], bf16,
                            kind="ExternalInput")
    Ah_t = nc.dram_tensor("Ah", [128, NT * NJ * 128], bf16, kind="ExternalInput")
    At_t = nc.dram_tensor("At", [128, NT * NJ * 128], bf16, kind="ExternalInput")
    gh_t = nc.dram_tensor("gh", [128, NJ], f32, kind="ExternalInput")
    gt_t = nc.dram_tensor("gt", [128, NJ], f32, kind="ExternalInput")
    rone_t = nc.dram_tensor("rone", [RELS, NJ * 128], f32, kind="ExternalInput")
    headi_t = nc.dram_tensor("headi", [128, NJ], i32, kind="ExternalInput")
    taili_t = nc.dram_tensor("taili", [128, NJ], i32, kind="ExternalInput")
    score_t = nc.dram_tensor("score", [128, NJ], f32, kind="ExternalOutput")

    # first/last (op) with groups, per T, for psum start/stop
    first_op = [min((op for op in range(NOP) if GT[T][op]), default=-1)
                for T in range(NT)]
    last_op = [max((op for op in range(NOP) if GT[T][op]), default=-1)
               for T in range(NT)]

    # split idx16 columns: first 3 ops early, rest later
    NHEAD = min(3, NOP)
    colsA = colo[NHEAD] if TOTC else 1
    colsB = max(TOTC - colsA, 0)

    with tile.TileContext(nc) as tc:
        with tc.tile_pool(name="const", bufs=1) as constp, \
             tc.tile_pool(name="old", bufs=1) as oldp, \
             tc.tile_pool(name="gath", bufs=8) as gathp, \
             tc.tile_pool(name="g16", bufs=8) as g16p, \
             tc.tile_pool(name="wld", bufs=8) as wldp, \
             tc.tile_pool(name="cpsum", bufs=1, space="PSUM") as cpsump, \
             tc.tile_pool(name="spsum", bufs=2, space="PSUM") as spsump, \
             tc.tile_pool(name="work", bufs=4) as workp:

            headi_sb = constp.tile([128, NJ], i32)
            nc.sync.dma_start(out=headi_sb[:], in_=headi_t.ap())
            taili_sb = constp.tile([128, NJ], i32)
            nc.sync.dma_start(out=taili_sb[:], in_=taili_t.ap())
            idxA = constp.tile([128, colsA], i16)
            nc.sync.dma_start(out=idxA[:], in_=idx16_t.ap()[:, :colsA])
            if colsB:
                idxB = constp.tile([128, colsB], i16)

            psts = [cpsump.tile([128, 128], f32, tag=f"ps{T}", name=f"ps{T}")
                    for T in range(NT)]
            old_h = [oldp.tile([128, D], f32, tag=f"oh{j}", name=f"oh{j}")
                     for j in range(NJ)]
            old_t = [oldp.tile([128, D], f32, tag=f"ot{j}", name=f"ot{j}")
                     for j in range(NJ)]
            dv16 = [constp.tile([128, D], bf16, tag=f"dv{T}", name=f"dv{T}")
                    for T in range(NT)]

            # ---- warm-up gather (pulls ext-isa IRAM load forward) ----
            warm = constp.tile([128, D], f32, tag="warm", name="warm")
            nc.gpsimd.dma_gather(
                out_ap=warm[:].rearrange("p (b d) -> p b d", d=D),
                in_ap=node_emb.ap()[0:BLK, :],
                idxs_ap=idxA[:, 0:1],
                num_idxs=16, num_idxs_reg=16, elem_size=D,
                single_packet=SP, queue_num=1)

            # ---- schedule: inds (q0) interleaved with comps (q1-3) ----
            items = []
            ji = 0
            inds = []
            for j in range(NJ):
                inds.append((j, "h"))
                inds.append((j, "t"))
            items.append(("ind", inds[0]))
            nq = [2, 3, 1]     # first comp after warm(q1) lands on q2
            for k in range(NOP):
                items.append(("comp", k))
                take = 1 if k < NOP - 1 else len(inds) - 1 - ji
                for _ in range(take):
                    ji += 1
                    if ji < len(inds):
                        items.append(("ind", inds[ji]))

            sync_loads = 0
            idxB_loaded = False
            for it in items:
                if it[0] == "ind":
                    j, hv = it[1]
                    dst = old_h[j] if hv == "h" else old_t[j]
                    src = headi_sb if hv == "h" else taili_sb
                    nc.gpsimd.indirect_dma_start(
                        out=dst[:], out_offset=None, in_=node_emb.ap(),
                        in_offset=bass.IndirectOffsetOnAxis(
                            ap=src[:, j:j + 1], axis=0))
                    continue
                k = it[1]
                bk = k                       # sched order == block order
                q = nq[k % 3]
                Gk = sum(GT[T][k] for T in range(NT))
                if Gk == 0:
                    continue
                ni = NI[k]
                c0, c1 = colo[k], colo[k + 1]
                if c1 <= colsA:
                    isrc = idxA[:, c0:c1]
                else:
                    isrc = idxB[:, c0 - colsA:c1 - colsA]
                gt_ = gathp.tile([128, Gmax * D], f32, tag="g")
                nc.gpsimd.dma_gather(
                    out_ap=gt_[:, :Gk * D].rearrange("p (b d) -> p b d", d=D),
                    in_ap=node_emb.ap()[bk * BLK:min((bk + 1) * BLK, N), :],
                    idxs_ap=isrc,
                    num_idxs=ni, num_idxs_reg=ni, elem_size=D,
                    single_packet=SP, queue_num=q)
                g16_ = g16p.tile([128, Gmax * D], bf16, tag="g16")
                nc.vector.tensor_copy(out=g16_[:, :Gk * D], in_=gt_[:, :Gk * D])
                wt_ = wldp.tile([128, Gmax * 128], bf16, tag="w")
                nc.sync.dma_start(
                    out=wt_[:, :Gk * 128],
                    in_=wmat_t.ap()[:, gofs[k] * 128:(gofs[k] + Gk) * 128])
                sync_loads += 1
                if sync_loads == NHEAD and colsB:
                    nc.sync.dma_start(out=idxB[:],
                                      in_=idx16_t.ap()[:, colsA:TOTC])
                    idxB_loaded = True
                g = 0
                for T in range(NT):
                    for gg in range(GT[T][k]):
                        nc.tensor.matmul(
                            out=psts[T][:],
                            lhsT=wt_[:, g * 128:(g + 1) * 128],
                            rhs=g16_[:, g * D:(g + 1) * D],
                            start=(k == first_op[T] and gg == 0),
                            stop=(k == last_op[T] and gg == GT[T][k] - 1))
                        g += 1
                for T in range(NT):
                    if k == last_op[T]:
                        nc.vector.tensor_copy(out=dv16[T][:], in_=psts[T][:])

            if colsB and not idxB_loaded:
                nc.sync.dma_start(out=idxB[:], in_=idx16_t.ap()[:, colsA:TOTC])
            for T in range(NT):
                if first_op[T] < 0:
                    nc.vector.memset(dv16[T][:], 0.0)

            Ah_sb = constp.tile([128, NT * NJ * 128], bf16)
            nc.sync.dma_start(out=Ah_sb[:], in_=Ah_t.ap())
            At_sb = constp.tile([128, NT * NJ * 128], bf16)
            nc.sync.dma_start(out=At_sb[:], in_=At_t.ap())
            gh_sb = constp.tile([128, NJ], f32)
            nc.sync.dma_start(out=gh_sb[:], in_=gh_t.ap())
            gt_sb = constp.tile([128, NJ], f32)
            nc.sync.dma_start(out=gt_sb[:], in_=gt_t.ap())
            rone_sb = constp.tile([RELS, NJ * 128], f32)
            nc.sync.dma_start(out=rone_sb[:], in_=rone_t.ap())
            rel_sb = constp.tile([RELS, D], f32)
            nc.sync.dma_start(out=rel_sb[:], in_=rel_emb.ap())

            # ---- score phase ----
            score_sb = constp.tile([128, NJ], f32)
            for j in range(NJ):
                ph = spsump.tile([128, 128], f32, tag="ph")
                pt = spsump.tile([128, 128], f32, tag="pt")
                pr = spsump.tile([128, 128], f32, tag="pr")
                for T in range(NT):
                    nc.tensor.matmul(
                        out=ph[:],
                        lhsT=Ah_sb[:, (T * NJ + j) * 128:(T * NJ + j + 1) * 128],
                        rhs=dv16[T][:], start=(T == 0), stop=(T == NT - 1))
                for T in range(NT):
                    nc.tensor.matmul(
                        out=pt[:],
                        lhsT=At_sb[:, (T * NJ + j) * 128:(T * NJ + j + 1) * 128],
                        rhs=dv16[T][:], start=(T == 0), stop=(T == NT - 1))
                nc.tensor.matmul(
                    out=pr[:], lhsT=rone_sb[:, j * 128:(j + 1) * 128],
                    rhs=rel_sb[:], start=True, stop=True)

                t2 = workp.tile([128, D], f32, tag="t2")
                nc.vector.tensor_scalar(
                    out=t2[:], in0=old_h[j][:], scalar1=gh_sb[:, j:j + 1],
                    scalar2=None, op0=Alu.mult)
                hv = workp.tile([128, D], f32, tag="hv")
                nc.vector.tensor_tensor(out=hv[:], in0=ph[:], in1=t2[:],
                                        op=Alu.add)
                t4 = workp.tile([128, D], f32, tag="t4")
                nc.vector.tensor_scalar(
                    out=t4[:], in0=old_t[j][:], scalar1=gt_sb[:, j:j + 1],
                    scalar2=None, op0=Alu.mult)
                tv = workp.tile([128, D], f32, tag="tv")
                nc.vector.tensor_tensor(out=tv[:], in0=pt[:], in1=t4[:],
                                        op=Alu.add)
                p1 = workp.tile([128, D], f32, tag="p1")
                nc.vector.tensor_tensor(out=p1[:], in0=hv[:], in1=tv[:],
                                        op=Alu.mult)
                p2 = workp.tile([128, D], f32, tag="p2")
                nc.vector.tensor_tensor(out=p2[:], in0=p1[:], in1=pr[:],
                                        op=Alu.mult)
                nc.vector.reduce_sum(out=score_sb[:, j:j + 1], in_=p2[:],
                                     axis=mybir.AxisListType.X)
            nc.sync.dma_start(out=score_t.ap(), in_=score_sb[:])

    nc.compile()
    return nc


def _get_nc(static):
    key = ("v8", static["NT"], tuple(map(tuple, static["GT"])),
           tuple(static["NI"]))
    if key not in _CACHE:
        _CACHE[key] = _build_nc(static)
    return _CACHE[key]


def kernel(head_index, rel_type, tail_index, node_emb, rel_emb,
           local_idx_map, sim_neighbors, sim_weights, degree_table):
    from concourse.bass_utils import run_bass_kernel_spmd

    head = np.asarray(head_index).astype(np.int64)
    rel = np.asarray(rel_type).astype(np.int64)
    tailv = np.asarray(tail_index).astype(np.int64)
    node_emb = np.ascontiguousarray(np.asarray(node_emb, dtype=np.float32))
    rel_emb = np.ascontiguousarray(np.asarray(rel_emb, dtype=np.float32))

    M = M_DEF
    while True:
        cores, extra = _prep_cores(head, rel, tailv, local_idx_map,
                                   sim_neighbors, sim_weights, degree_table, M)
        if cores != "slots":
            break
        M *= 2
    static, perm = extra

    nc = _get_nc(static)
    in_maps = []
    for c in range(NCORES):
        cc = cores[c]
        in_maps.append({
            "node_emb": node_emb, "rel_emb": rel_emb,
            "idx16": cc["idx16"], "wmat": cc["wmat"],
            "Ah": cc["Ah"], "At": cc["At"],
            "gh": cc["gh"], "gt": cc["gt"], "rone": cc["rone"],
            "headi": cc["headi"], "taili": cc["taili"],
        })

    _CACHE["last_in_maps"] = in_maps
    res = run_bass_kernel_spmd(nc, in_maps, core_ids=list(range(NCORES)))
    _CACHE["last_result"] = res
    _CACHE["last_nc"] = nc
    _CACHE["last_perm"] = perm

    out = np.empty(B, np.float32)
    for c in range(NCORES):
        out[perm[c]] = res.results[c]["score"].T.reshape(-1)
    return out


# revision 9
# speedup vs baseline: 1.1382x; 1.1382x over previous
"""Trainium2 Bass kernel for nn_DistMultMod, v9.

Per core (BC=1024 triplets, balanced assignment):
  - comp slots (masked last-writer nodes, ~195/core): dv = sum_k w*node_emb[neigh]
    via dma_gather of neighbor rows bucketed by 32768-row block (16 merged ops,
    [T0 cap | T1 cap] sections, pad idx 0 w=0), + PE matmuls with host-built
    scaled one-hot W (bf16) accumulating into per-parity PSUM tiles.
  - old head/tail rows: ONE multi-column indirect DMA (offsets [128, 2*NJ])
    into old_all [128, 2*NJ*D].
  - score: per j-tile: psum_h = sum_T Ah[T,j].T @ dv16[T]; pr = Rone.T @ rel;
    h = psum_h + gh*old_h; t = psum_t + gt*old_t; score = sum(h*t*pr).

v9 vs v7 (v8 regressed: ind/comp interleave serialized emission because
DMA_INDIRECT descriptor-gen uses all 8 Q7 cores; the ext-isa lib load took
25us mid-schedule):
  - explicit gpsimd.load_library(mlp) first: IRAM load at t~0.5us.
  - one multicol indirect for all 2048 old rows (replaces 16 serial ops).
  - comps contiguous on queues 0-3 (no inds between them -> 3-4x overlap).
  - idx16 split: first 4 ops' columns load first; Ah/At/rone/rel deferred.
  - balanced triplet->core assignment (writer triplets dealt round-robin).
"""
import numpy as np
import ml_dtypes

BF16 = ml_dtypes.bfloat16

B = 8192
NCORES = 8
BC = B // NCORES
D = 128
K = 64
N = 500000
RELS = 16
ND = 20000
NBLK = 16
BLK = 32768
NJ = BC // 128
LAM = 0.7
SP = False
M_DEF = 256
GCAP_DEF = 4
MULTI_IND = False        # one [128, 2*NJ]-offset indirect vs per-j ops
                         # (multicol indirect produced wrong results on HW)

_CACHE = {}


def _assign_cores(head, rel, mask):
    last_of = {}
    for b in range(B):
        last_of[int(head[b])] = b
    is_writer = np.zeros(B, bool)
    for b in range(B):
        if mask[b] and last_of[int(head[b])] == b:
            is_writer[b] = True
    perm = [[] for _ in range(NCORES)]
    c = 0
    for b in np.flatnonzero(is_writer):
        perm[c % NCORES].append(int(b))
        c += 1
    c = 0
    for b in np.flatnonzero(~is_writer):
        while len(perm[c % NCORES]) >= BC:
            c += 1
        perm[c % NCORES].append(int(b))
        c += 1
    return np.array(perm, np.int64), last_of


def _prep_cores(head, rel, tailv, local_idx_map, sim_neighbors, sim_weights,
                degree_table, M, GCAP):
    NT = M // 128
    CAP = GCAP * 128                 # rows per (block, T) section
    mask = (rel >= 2) & (rel <= 4)
    local_idx_map = np.asarray(local_idx_map)
    sim_neighbors = np.asarray(sim_neighbors)
    sim_weights = np.asarray(sim_weights)
    degree_table = np.asarray(degree_table)

    perm, last_of = _assign_cores(head, rel, mask)

    raw = []
    for c in range(NCORES):
        pc = perm[c]
        slot_of = {}
        writers = []

        def slot_for(node):
            bw = last_of.get(int(node), -1)
            if bw < 0 or not mask[bw]:
                return M - 1
            s = slot_of.get(bw, -1)
            if s < 0:
                s = len(writers)
                slot_of[bw] = s
                writers.append(bw)
            return s

        slot_h = np.empty(BC, np.int32)
        slot_t = np.empty(BC, np.int32)
        for i in range(BC):
            slot_h[i] = slot_for(head[pc[i]])
            slot_t[i] = slot_for(tailv[pc[i]])
        m = len(writers)
        if m > M - 1:
            return "slots", None

        a = np.zeros(M, np.float32)
        wl = np.array(writers, dtype=np.int64)
        if m:
            ls = local_idx_map[head[wl]]
            neigh_rows = sim_neighbors[ls].astype(np.int64)
            w_rows = sim_weights[ls].astype(np.float32)
            deg = degree_table[ls, rel[wl] - 2].astype(np.float32)
            a[:m] = LAM * np.exp(-LAM * deg) + 0.2
        else:
            neigh_rows = np.zeros((0, K), np.int64)
            w_rows = np.zeros((0, K), np.float32)

        srows = np.repeat(np.arange(m), K)
        rnodes = neigh_rows.reshape(-1)
        wvals = w_rows.reshape(-1)
        Tpar = (srows % NT).astype(np.int64)
        blk = rnodes >> 15
        pcol = (srows // NT).astype(np.int64)
        off = (rnodes & (BLK - 1)).astype(np.int64)
        order = np.lexsort((pcol, Tpar, blk))
        raw.append(dict(slot_h=slot_h, slot_t=slot_t, a=a, pc=pc,
                        sTs=Tpar[order], sblk=blk[order], soff=off[order],
                        sw=wvals[order], sp=pcol[order]))

    cnt = np.zeros((NCORES, NBLK, NT), np.int64)
    for c in range(NCORES):
        r = raw[c]
        np.add.at(cnt[c], (r["sblk"], r["sTs"]), 1)
    if cnt.max() > CAP:
        return "bucket", None

    GT = [[GCAP] * NBLK for _ in range(NT)]
    NI = [NT * CAP] * NBLK
    static = dict(NT=NT, GT=GT, NI=NI, M=M, Gmax=NT * GCAP)

    cores = []
    for c in range(NCORES):
        r = raw[c]
        a, slot_h, slot_t, pc = r["a"], r["slot_h"], r["slot_t"], r["pc"]
        idx_cols = []
        wgroups = []
        for op in range(NBLK):
            for T in range(NT):
                sel = (r["sblk"] == op) & (r["sTs"] == T)
                offs = r["soff"][sel]
                ws = r["sw"][sel]
                ps = r["sp"][sel]
                n = len(offs)
                idx = np.zeros(CAP, np.int16)
                idx[:n] = offs
                W = np.zeros((GCAP, 128, 128), np.float32)
                pos = np.arange(n)
                W[pos // 128, pos % 128, ps] = ws
                idx_cols.append(idx)
                wgroups.append(W.reshape(CAP, 128))
        idx_all = np.concatenate(idx_cols)
        tc = len(idx_all) // 16
        idx16 = np.tile(idx_all.reshape(tc, 16).T, (8, 1))
        wmat = np.concatenate(wgroups, axis=0)
        wmat = np.ascontiguousarray(
            wmat.reshape(-1, 128, 128).transpose(1, 0, 2)
            .reshape(128, -1)).astype(BF16)

        i = np.arange(BC)
        j_, t_ = i // 128, i % 128
        Ah = np.zeros((128, NT * NJ * 128), np.float32)
        T_, p_ = slot_h % NT, slot_h // NT
        Ah[p_, (T_ * NJ + j_) * 128 + t_] = a[slot_h]
        At = np.zeros((128, NT * NJ * 128), np.float32)
        T_, p_ = slot_t % NT, slot_t // NT
        At[p_, (T_ * NJ + j_) * 128 + t_] = a[slot_t]
        gh = np.ascontiguousarray((1.0 - a[slot_h]).reshape(NJ, 128).T)
        gt = np.ascontiguousarray((1.0 - a[slot_t]).reshape(NJ, 128).T)
        rone = np.zeros((RELS, NJ * 128), np.float32)
        rone[rel[pc], i] = 1.0

        hmat = head[pc].reshape(NJ, 128).T      # [128, NJ]
        tmat = tailv[pc].reshape(NJ, 128).T
        headtail = np.concatenate([hmat, tmat], axis=1)  # [128, 2*NJ]

        cores.append(dict(
            idx16=np.ascontiguousarray(idx16),
            wmat=wmat,
            Ah=np.ascontiguousarray(Ah).astype(BF16),
            At=np.ascontiguousarray(At).astype(BF16),
            gh=gh.astype(np.float32),
            gt=gt.astype(np.float32),
            rone=np.ascontiguousarray(rone),
            headtail=np.ascontiguousarray(headtail).astype(np.int32),
        ))
    return cores, (static, perm)


def _build_nc(static):
    import concourse.bass as bass
    import concourse.bacc as bacc
    import concourse.mybir as mybir
    import concourse.tile as tile
    from concourse import library_config

    NT = static["NT"]
    GT = static["GT"]
    NI = static["NI"]
    Gmax = static["Gmax"]
    NOP = len(NI)
    f32 = mybir.dt.float32
    bf16 = mybir.dt.bfloat16
    i32 = mybir.dt.int32
    i16 = mybir.dt.int16
    Alu = mybir.AluOpType

    TOTC = sum(ni // 16 for ni in NI)
    colo = np.cumsum([0] + [ni // 16 for ni in NI]).tolist()
    gofs = []
    acc = 0
    for op in range(NOP):
        gofs.append(acc)
        acc += sum(GT[T][op] for T in range(NT))
    TOTG = acc

    nc = bacc.Bacc("TRN2", target_bir_lowering=False, debug=False,
                   num_devices=NCORES, num_swdge_queues=4)

    node_emb = nc.dram_tensor("node_emb", [N, D], f32, kind="ExternalInput")
    rel_emb = nc.dram_tensor("rel_emb", [RELS, D], f32, kind="ExternalInput")
    idx16_t = nc.dram_tensor("idx16", [128, TOTC], i16, kind="ExternalInput")
    wmat_t = nc.dram_tensor("wmat", [128, TOTG * 128], bf16,
                            kind="ExternalInput")
    Ah_t = nc.dram_tensor("Ah", [128, NT * NJ * 128], bf16, kind="ExternalInput")
    At_t = nc.dram_tensor("At", [128, NT * NJ * 128], bf16, kind="ExternalInput")
    gh_t = nc.dram_tensor("gh", [128, NJ], f32, kind="ExternalInput")
    gt_t = nc.dram_tensor("gt", [128, NJ], f32, kind="ExternalInput")
    rone_t = nc.dram_tensor("rone", [RELS, NJ * 128], f32, kind="ExternalInput")
    ht_t = nc.dram_tensor("headtail", [128, 2 * NJ], i32, kind="ExternalInput")
    score_t = nc.dram_tensor("score", [128, NJ], f32, kind="ExternalOutput")

    first_op = [min((op for op in range(NOP) if GT[T][op]), default=-1)
                for T in range(NT)]
    last_op = [max((op for op in range(NOP) if GT[T][op]), default=-1)
               for T in range(NT)]

    NHEAD = min(4, NOP)
    colsA = colo[NHEAD]
    colsB = TOTC - colsA

    with tile.TileContext(nc) as tc:
        with tc.tile_pool(name="const", bufs=1) as constp, \
             tc.tile_pool(name="old", bufs=1) as oldp, \
             tc.tile_pool(name="gath", bufs=8) as gathp, \
             tc.tile_pool(name="g16", bufs=8) as g16p, \
             tc.tile_pool(name="wld", bufs=8) as wldp, \
             tc.tile_pool(name="cpsum", bufs=1, space="PSUM") as cpsump, \
             tc.tile_pool(name="spsum", bufs=2, space="PSUM") as spsump, \
             tc.tile_pool(name="work", bufs=4) as workp:

            nc.gpsimd.load_library(library_config.mlp)

            ht_sb = constp.tile([128, 2 * NJ], i32)
            nc.sync.dma_start(out=ht_sb[:], in_=ht_t.ap())
            idxA = constp.tile([128, colsA], i16)
            nc.sync.dma_start(out=idxA[:], in_=idx16_t.ap()[:, :colsA])
            if colsB:
                idxB = constp.tile([128, colsB], i16)

            psts = [cpsump.tile([128, 128], f32, tag=f"ps{T}", name=f"ps{T}")
                    for T in range(NT)]
            dv16 = [constp.tile([128, D], bf16, tag=f"dv{T}", name=f"dv{T}")
                    for T in range(NT)]

            # ---- old head/tail rows ----
            if MULTI_IND:
                old_all = oldp.tile([128, 2 * NJ * D], f32, tag="oldall",
                                    name="oldall")
                nc.gpsimd.indirect_dma_start(
                    out=old_all[:], out_offset=None, in_=node_emb.ap(),
                    in_offset=bass.IndirectOffsetOnAxis(
                        ap=ht_sb[:, 0:2 * NJ], axis=0))

                def old_h(j):
                    return old_all[:, j * D:(j + 1) * D]

                def old_t(j):
                    return old_all[:, (NJ + j) * D:(NJ + j + 1) * D]
            else:
                oh_tiles = [oldp.tile([128, D], f32, tag=f"oh{j}",
                                      name=f"oh{j}") for j in range(NJ)]
                ot_tiles = [oldp.tile([128, D], f32, tag=f"ot{j}",
                                      name=f"ot{j}") for j in range(NJ)]
                for j in range(NJ):
                    nc.gpsimd.indirect_dma_start(
                        out=oh_tiles[j][:], out_offset=None,
                        in_=node_emb.ap(),
                        in_offset=bass.IndirectOffsetOnAxis(
                            ap=ht_sb[:, j:j + 1], axis=0))
                    nc.gpsimd.indirect_dma_start(
                        out=ot_tiles[j][:], out_offset=None,
                        in_=node_emb.ap(),
                        in_offset=bass.IndirectOffsetOnAxis(
                            ap=ht_sb[:, NJ + j:NJ + j + 1], axis=0))

                def old_h(j):
                    return oh_tiles[j][:]

                def old_t(j):
                    return ot_tiles[j][:]

            # ---- comp gathers: contiguous, queues 0-3 ----
            sync_loads = 0
            idxB_loaded = False
            for k in range(NOP):
                Gk = sum(GT[T][k] for T in range(NT))
                if Gk == 0:
                    continue
                ni = NI[k]
                c0, c1 = colo[k], colo[k + 1]
                if c1 <= colsA:
                    isrc = idxA[:, c0:c1]
                else:
                    isrc = idxB[:, c0 - colsA:c1 - colsA]
                gt_ = gathp.tile([128, Gmax * D], f32, tag="g")
                nc.gpsimd.dma_gather(
                    out_ap=gt_[:, :Gk * D].rearrange("p (b d) -> p b d", d=D),
                    in_ap=node_emb.ap()[k * BLK:min((k + 1) * BLK, N), :],
                    idxs_ap=isrc,
                    num_idxs=ni, num_idxs_reg=ni, elem_size=D,
                    single_packet=SP, queue_num=k % 4)
                g16_ = g16p.tile([128, Gmax * D], bf16, tag="g16")
                nc.vector.tensor_copy(out=g16_[:, :Gk * D], in_=gt_[:, :Gk * D])
                wt_ = wldp.tile([128, Gmax * 128], bf16, tag="w")
                nc.sync.dma_start(
                    out=wt_[:, :Gk * 128],
                    in_=wmat_t.ap()[:, gofs[k] * 128:(gofs[k] + Gk) * 128])
                sync_loads += 1
                if sync_loads == NHEAD and colsB:
                    nc.sync.dma_start(out=idxB[:],
                                      in_=idx16_t.ap()[:, colsA:TOTC])
                    idxB_loaded = True
                g = 0
                for T in range(NT):
                    for gg in range(GT[T][k]):
                        nc.tensor.matmul(
                            out=psts[T][:],
                            lhsT=wt_[:, g * 128:(g + 1) * 128],
                            rhs=g16_[:, g * D:(g + 1) * D],
                            start=(k == first_op[T] and gg == 0),
                            stop=(k == last_op[T] and gg == GT[T][k] - 1))
                        g += 1
                for T in range(NT):
                    if k == last_op[T]:
                        nc.vector.tensor_copy(out=dv16[T][:], in_=psts[T][:])

            if colsB and not idxB_loaded:
                nc.sync.dma_start(out=idxB[:], in_=idx16_t.ap()[:, colsA:TOTC])
            for T in range(NT):
                if first_op[T] < 0:
                    nc.vector.memset(dv16[T][:], 0.0)

            Ah_sb = constp.tile([128, NT * NJ * 128], bf16)
            nc.sync.dma_start(out=Ah_sb[:], in_=Ah_t.ap())
            At_sb = constp.tile([128, NT * NJ * 128], bf16)
            nc.sync.dma_start(out=At_sb[:], in_=At_t.ap())
            gh_sb = constp.tile([128, NJ], f32)
            nc.sync.dma_start(out=gh_sb[:], in_=gh_t.ap())
            gt_sb = constp.tile([128, NJ], f32)
            nc.sync.dma_start(out=gt_sb[:], in_=gt_t.ap())
            rone_sb = constp.tile([RELS, NJ * 128], f32)
            nc.sync.dma_start(out=rone_sb[:], in_=rone_t.ap())
            rel_sb = constp.tile([RELS, D], f32)
            nc.sync.dma_start(out=rel_sb[:], in_=rel_emb.ap())

            # ---- score phase ----
            score_sb = constp.tile([128, NJ], f32)
            for j in range(NJ):
                ph = spsump.tile([128, 128], f32, tag="ph")
                pt = spsump.tile([128, 128], f32, tag="pt")
                pr = spsump.tile([128, 128], f32, tag="pr")
                for T in range(NT):
                    nc.tensor.matmul(
                        out=ph[:],
                        lhsT=Ah_sb[:, (T * NJ + j) * 128:(T * NJ + j + 1) * 128],
                        rhs=dv16[T][:], start=(T == 0), stop=(T == NT - 1))
                for T in range(NT):
                    nc.tensor.matmul(
                        out=pt[:],
                        lhsT=At_sb[:, (T * NJ + j) * 128:(T * NJ + j + 1) * 128],
                        rhs=dv16[T][:], start=(T == 0), stop=(T == NT - 1))
                nc.tensor.matmul(
                    out=pr[:], lhsT=rone_sb[:, j * 128:(j + 1) * 128],
                    rhs=rel_sb[:], start=True, stop=True)

                t2 = workp.tile([128, D], f32, tag="t2")
                nc.vector.tensor_scalar(
                    out=t2[:], in0=old_h(j), scalar1=gh_sb[:, j:j + 1],
                    scalar2=None, op0=Alu.mult)
                hv = workp.tile([128, D], f32, tag="hv")
                nc.vector.tensor_tensor(out=hv[:], in0=ph[:], in1=t2[:],
                                        op=Alu.add)
                t4 = workp.tile([128, D], f32, tag="t4")
                nc.vector.tensor_scalar(
                    out=t4[:], in0=old_t(j), scalar1=gt_sb[:, j:j + 1],
                    scalar2=None, op0=Alu.mult)
                tv = workp.tile([128, D], f32, tag="tv")
                nc.vector.tensor_tensor(out=tv[:], in0=pt[:], in1=t4[:],
                                        op=Alu.add)
                p1 = workp.tile([128, D], f32, tag="p1")
                nc.vector.tensor_tensor(out=p1[:], in0=hv[:], in1=tv[:],
                                        op=Alu.mult)
                p2 = workp.tile([128, D], f32, tag="p2")
                nc.vector.tensor_tensor(out=p2[:], in0=p1[:], in1=pr[:],
                                        op=Alu.mult)
                nc.vector.reduce_sum(out=score_sb[:, j:j + 1], in_=p2[:],
                                     axis=mybir.AxisListType.X)
            nc.sync.dma_start(out=score_t.ap(), in_=score_sb[:])

    nc.compile()
    return nc


def _get_nc(static):
    key = ("v9", MULTI_IND, static["NT"], tuple(map(tuple, static["GT"])),
           tuple(static["NI"]))
    if key not in _CACHE:
        _CACHE[key] = _build_nc(static)
    return _CACHE[key]


def kernel(head_index, rel_type, tail_index, node_emb, rel_emb,
           local_idx_map, sim_neighbors, sim_weights, degree_table):
    from concourse.bass_utils import run_bass_kernel_spmd

    head = np.asarray(head_index).astype(np.int64)
    rel = np.asarray(rel_type).astype(np.int64)
    tailv = np.asarray(tail_index).astype(np.int64)
    node_emb = np.ascontiguousarray(np.asarray(node_emb, dtype=np.float32))
    rel_emb = np.ascontiguousarray(np.asarray(rel_emb, dtype=np.float32))

    M, GCAP = M_DEF, GCAP_DEF
    while True:
        cores, extra = _prep_cores(head, rel, tailv, local_idx_map,
                                   sim_neighbors, sim_weights, degree_table,
                                   M, GCAP)
        if cores == "slots":
            M *= 2
        elif cores == "bucket":
            GCAP += 1
        else:
            break
    static, perm = extra

    nc = _get_nc(static)
    in_maps = []
    for c in range(NCORES):
        cc = cores[c]
        in_maps.append({
            "node_emb": node_emb, "rel_emb": rel_emb,
            "idx16": cc["idx16"], "wmat": cc["wmat"],
            "Ah": cc["Ah"], "At": cc["At"],
            "gh": cc["gh"], "gt": cc["gt"], "rone": cc["rone"],
            "headtail": cc["headtail"],
        })

    _CACHE["last_in_maps"] = in_maps
    res = run_bass_kernel_spmd(nc, in_maps, core_ids=list(range(NCORES)))
    _CACHE["last_result"] = res
    _CACHE["last_nc"] = nc
    _CACHE["last_perm"] = perm

    out = np.empty(B, np.float32)
    for c in range(NCORES):
        out[perm[c]] = res.results[c]["score"].T.reshape(-1)
    return out


# revision 12
# speedup vs baseline: 1.4082x; 1.2373x over previous
"""Trainium2 Bass kernel for nn_DistMultMod, v9.

Per core (BC=1024 triplets, balanced assignment):
  - comp slots (masked last-writer nodes, ~195/core): dv = sum_k w*node_emb[neigh]
    via dma_gather of neighbor rows bucketed by 32768-row block (16 merged ops,
    [T0 cap | T1 cap] sections, pad idx 0 w=0), + PE matmuls with host-built
    scaled one-hot W (bf16) accumulating into per-parity PSUM tiles.
  - old head/tail rows: ONE multi-column indirect DMA (offsets [128, 2*NJ])
    into old_all [128, 2*NJ*D].
  - score: per j-tile: psum_h = sum_T Ah[T,j].T @ dv16[T]; pr = Rone.T @ rel;
    h = psum_h + gh*old_h; t = psum_t + gt*old_t; score = sum(h*t*pr).

v9 vs v7 (v8 regressed: ind/comp interleave serialized emission because
DMA_INDIRECT descriptor-gen uses all 8 Q7 cores; the ext-isa lib load took
25us mid-schedule):
  - explicit gpsimd.load_library(mlp) first: IRAM load at t~0.5us.
  - one multicol indirect for all 2048 old rows (replaces 16 serial ops).
  - comps contiguous on queues 0-3 (no inds between them -> 3-4x overlap).
  - idx16 split: first 4 ops' columns load first; Ah/At/rone/rel deferred.
  - balanced triplet->core assignment (writer triplets dealt round-robin).
"""
import numpy as np
import ml_dtypes

BF16 = ml_dtypes.bfloat16

B = 8192
NCORES = 8
BC = B // NCORES
D = 128
K = 64
N = 500000
RELS = 16
ND = 20000
NBLK = 16
BLK = 32768
NJ = BC // 128
LAM = 0.7
SP = True
M_DEF = 256
GCAP_DEF = 4
MULTI_IND = False        # one [128, 2*NJ]-offset indirect vs per-j ops
                         # (multicol indirect produced wrong results on HW)

_CACHE = {}


def _assign_cores(head, rel, mask):
    last_of = {}
    for b in range(B):
        last_of[int(head[b])] = b
    is_writer = np.zeros(B, bool)
    for b in range(B):
        if mask[b] and last_of[int(head[b])] == b:
            is_writer[b] = True
    perm = [[] for _ in range(NCORES)]
    c = 0
    for b in np.flatnonzero(is_writer):
        perm[c % NCORES].append(int(b))
        c += 1
    c = 0
    for b in np.flatnonzero(~is_writer):
        while len(perm[c % NCORES]) >= BC:
            c += 1
        perm[c % NCORES].append(int(b))
        c += 1
    return np.array(perm, np.int64), last_of


def _prep_cores(head, rel, tailv, local_idx_map, sim_neighbors, sim_weights,
                degree_table, M, GCAP):
    NT = M // 128
    CAP = GCAP * 128                 # rows per (block, T) section
    mask = (rel >= 2) & (rel <= 4)
    local_idx_map = np.asarray(local_idx_map)
    sim_neighbors = np.asarray(sim_neighbors)
    sim_weights = np.asarray(sim_weights)
    degree_table = np.asarray(degree_table)

    perm, last_of = _assign_cores(head, rel, mask)

    raw = []
    for c in range(NCORES):
        pc = perm[c]
        slot_of = {}
        writers = []

        def slot_for(node):
            bw = last_of.get(int(node), -1)
            if bw < 0 or not mask[bw]:
                return M - 1
            s = slot_of.get(bw, -1)
            if s < 0:
                s = len(writers)
                slot_of[bw] = s
                writers.append(bw)
            return s

        slot_h = np.empty(BC, np.int32)
        slot_t = np.empty(BC, np.int32)
        for i in range(BC):
            slot_h[i] = slot_for(head[pc[i]])
            slot_t[i] = slot_for(tailv[pc[i]])
        m = len(writers)
        if m > M - 1:
            return "slots", None

        a = np.zeros(M, np.float32)
        wl = np.array(writers, dtype=np.int64)
        if m:
            ls = local_idx_map[head[wl]]
            neigh_rows = sim_neighbors[ls].astype(np.int64)
            w_rows = sim_weights[ls].astype(np.float32)
            deg = degree_table[ls, rel[wl] - 2].astype(np.float32)
            a[:m] = LAM * np.exp(-LAM * deg) + 0.2
        else:
            neigh_rows = np.zeros((0, K), np.int64)
            w_rows = np.zeros((0, K), np.float32)

        srows = np.repeat(np.arange(m), K)
        rnodes = neigh_rows.reshape(-1)
        wvals = w_rows.reshape(-1)
        Tpar = (srows % NT).astype(np.int64)
        blk = rnodes >> 15
        pcol = (srows // NT).astype(np.int64)
        off = (rnodes & (BLK - 1)).astype(np.int64)
        order = np.lexsort((pcol, Tpar, blk))
        raw.append(dict(slot_h=slot_h, slot_t=slot_t, a=a, pc=pc,
                        sTs=Tpar[order], sblk=blk[order], soff=off[order],
                        sw=wvals[order], sp=pcol[order]))

    cnt = np.zeros((NCORES, NBLK, NT), np.int64)
    for c in range(NCORES):
        r = raw[c]
        np.add.at(cnt[c], (r["sblk"], r["sTs"]), 1)
    if cnt.max() > CAP:
        return "bucket", None

    GT = [[GCAP] * NBLK for _ in range(NT)]
    NI = [NT * CAP] * NBLK
    static = dict(NT=NT, GT=GT, NI=NI, M=M, Gmax=NT * GCAP)

    cores = []
    for c in range(NCORES):
        r = raw[c]
        a, slot_h, slot_t, pc = r["a"], r["slot_h"], r["slot_t"], r["pc"]
        idx_cols = []
        wgroups = []
        for op in range(NBLK):
            for T in range(NT):
                sel = (r["sblk"] == op) & (r["sTs"] == T)
                offs = r["soff"][sel]
                ws = r["sw"][sel]
                ps = r["sp"][sel]
                n = len(offs)
                idx = np.zeros(CAP, np.int16)
                idx[:n] = offs
                W = np.zeros((GCAP, 128, 128), np.float32)
                pos = np.arange(n)
                W[pos // 128, pos % 128, ps] = ws
                idx_cols.append(idx)
                wgroups.append(W.reshape(CAP, 128))
        idx_all = np.concatenate(idx_cols)
        tc = len(idx_all) // 16
        idx16 = np.tile(idx_all.reshape(tc, 16).T, (8, 1))
        wmat = np.concatenate(wgroups, axis=0)
        wmat = np.ascontiguousarray(
            wmat.reshape(-1, 128, 128).transpose(1, 0, 2)
            .reshape(128, -1)).astype(BF16)

        i = np.arange(BC)
        j_, t_ = i // 128, i % 128
        Ah = np.zeros((128, NT * NJ * 128), np.float32)
        T_, p_ = slot_h % NT, slot_h // NT
        Ah[p_, (T_ * NJ + j_) * 128 + t_] = a[slot_h]
        At = np.zeros((128, NT * NJ * 128), np.float32)
        T_, p_ = slot_t % NT, slot_t // NT
        At[p_, (T_ * NJ + j_) * 128 + t_] = a[slot_t]
        gh = np.ascontiguousarray((1.0 - a[slot_h]).reshape(NJ, 128).T)
        gt = np.ascontiguousarray((1.0 - a[slot_t]).reshape(NJ, 128).T)
        rone = np.zeros((RELS, NJ * 128), np.float32)
        rone[rel[pc], i] = 1.0

        hmat = head[pc].reshape(NJ, 128).T      # [128, NJ]
        tmat = tailv[pc].reshape(NJ, 128).T
        headtail = np.concatenate([hmat, tmat], axis=1)  # [128, 2*NJ]

        cores.append(dict(
            idx16=np.ascontiguousarray(idx16),
            wmat=wmat,
            Ah=np.ascontiguousarray(Ah).astype(BF16),
            At=np.ascontiguousarray(At).astype(BF16),
            gh=gh.astype(np.float32),
            gt=gt.astype(np.float32),
            rone=np.ascontiguousarray(rone),
            headtail=np.ascontiguousarray(headtail).astype(np.int32),
        ))
    return cores, (static, perm)


def _build_nc(static):
    import concourse.bass as bass
    import concourse.bacc as bacc
    import concourse.mybir as mybir
    import concourse.tile as tile
    from concourse import library_config

    NT = static["NT"]
    GT = static["GT"]
    NI = static["NI"]
    Gmax = static["Gmax"]
    NOP = len(NI)
    f32 = mybir.dt.float32
    bf16 = mybir.dt.bfloat16
    i32 = mybir.dt.int32
    i16 = mybir.dt.int16
    Alu = mybir.AluOpType

    TOTC = sum(ni // 16 for ni in NI)
    colo = np.cumsum([0] + [ni // 16 for ni in NI]).tolist()
    gofs = []
    acc = 0
    for op in range(NOP):
        gofs.append(acc)
        acc += sum(GT[T][op] for T in range(NT))
    TOTG = acc

    nc = bacc.Bacc("TRN2", target_bir_lowering=False, debug=False,
                   num_devices=NCORES, num_swdge_queues=4,
                   dynamic_dma_scratch_size=65536)

    node_emb = nc.dram_tensor("node_emb", [N, D], f32, kind="ExternalInput")
    rel_emb = nc.dram_tensor("rel_emb", [RELS, D], f32, kind="ExternalInput")
    idx16_t = nc.dram_tensor("idx16", [128, TOTC], i16, kind="ExternalInput")
    wmat_t = nc.dram_tensor("wmat", [128, TOTG * 128], bf16,
                            kind="ExternalInput")
    Ah_t = nc.dram_tensor("Ah", [128, NT * NJ * 128], bf16, kind="ExternalInput")
    At_t = nc.dram_tensor("At", [128, NT * NJ * 128], bf16, kind="ExternalInput")
    gh_t = nc.dram_tensor("gh", [128, NJ], f32, kind="ExternalInput")
    gt_t = nc.dram_tensor("gt", [128, NJ], f32, kind="ExternalInput")
    rone_t = nc.dram_tensor("rone", [RELS, NJ * 128], f32, kind="ExternalInput")
    ht_t = nc.dram_tensor("headtail", [128, 2 * NJ], i32, kind="ExternalInput")
    score_t = nc.dram_tensor("score", [128, NJ], f32, kind="ExternalOutput")

    first_op = [min((op for op in range(NOP) if GT[T][op]), default=-1)
                for T in range(NT)]
    last_op = [max((op for op in range(NOP) if GT[T][op]), default=-1)
               for T in range(NT)]

    NHEAD = min(4, NOP)
    colsA = colo[NHEAD]
    colsB = TOTC - colsA

    with tile.TileContext(nc) as tc:
        with tc.tile_pool(name="const", bufs=1) as constp, \
             tc.tile_pool(name="old", bufs=1) as oldp, \
             tc.tile_pool(name="gath", bufs=8) as gathp, \
             tc.tile_pool(name="g16", bufs=8) as g16p, \
             tc.tile_pool(name="wld", bufs=8) as wldp, \
             tc.tile_pool(name="cpsum", bufs=1, space="PSUM") as cpsump, \
             tc.tile_pool(name="spsum", bufs=2, space="PSUM") as spsump, \
             tc.tile_pool(name="work", bufs=4) as workp:

            nc.gpsimd.load_library(library_config.mlp)

            ht_sb = constp.tile([128, 2 * NJ], i32)
            nc.sync.dma_start(out=ht_sb[:], in_=ht_t.ap())
            idxA = constp.tile([128, colsA], i16)
            nc.sync.dma_start(out=idxA[:], in_=idx16_t.ap()[:, :colsA])
            if colsB:
                idxB = constp.tile([128, colsB], i16)

            psts = [cpsump.tile([128, 128], f32, tag=f"ps{T}", name=f"ps{T}")
                    for T in range(NT)]
            dv16 = [constp.tile([128, D], bf16, tag=f"dv{T}", name=f"dv{T}")
                    for T in range(NT)]

            # ---- old head/tail rows ----
            if MULTI_IND:
                old_all = oldp.tile([128, 2 * NJ * D], f32, tag="oldall",
                                    name="oldall")
                nc.gpsimd.indirect_dma_start(
                    out=old_all[:], out_offset=None, in_=node_emb.ap(),
                    in_offset=bass.IndirectOffsetOnAxis(
                        ap=ht_sb[:, 0:2 * NJ], axis=0))

                def old_h(j):
                    return old_all[:, j * D:(j + 1) * D]

                def old_t(j):
                    return old_all[:, (NJ + j) * D:(NJ + j + 1) * D]
            else:
                oh_tiles = [oldp.tile([128, D], f32, tag=f"oh{j}",
                                      name=f"oh{j}") for j in range(NJ)]
                ot_tiles = [oldp.tile([128, D], f32, tag=f"ot{j}",
                                      name=f"ot{j}") for j in range(NJ)]
                for j in range(NJ):
                    nc.gpsimd.indirect_dma_start(
                        out=oh_tiles[j][:], out_offset=None,
                        in_=node_emb.ap(),
                        in_offset=bass.IndirectOffsetOnAxis(
                            ap=ht_sb[:, j:j + 1], axis=0))
                    nc.gpsimd.indirect_dma_start(
                        out=ot_tiles[j][:], out_offset=None,
                        in_=node_emb.ap(),
                        in_offset=bass.IndirectOffsetOnAxis(
                            ap=ht_sb[:, NJ + j:NJ + j + 1], axis=0))

                def old_h(j):
                    return oh_tiles[j][:]

                def old_t(j):
                    return ot_tiles[j][:]

            # ---- comp gathers: contiguous, queues 0-3 ----
            sync_loads = 0
            idxB_loaded = False
            for k in range(NOP):
                Gk = sum(GT[T][k] for T in range(NT))
                if Gk == 0:
                    continue
                ni = NI[k]
                c0, c1 = colo[k], colo[k + 1]
                if c1 <= colsA:
                    isrc = idxA[:, c0:c1]
                else:
                    isrc = idxB[:, c0 - colsA:c1 - colsA]
                gt_ = gathp.tile([128, Gmax * D], f32, tag="g")
                nc.gpsimd.dma_gather(
                    out_ap=gt_[:, :Gk * D].rearrange("p (b d) -> p b d", d=D),
                    in_ap=node_emb.ap()[k * BLK:min((k + 1) * BLK, N), :],
                    idxs_ap=isrc,
                    num_idxs=ni, num_idxs_reg=ni, elem_size=D,
                    single_packet=SP, queue_num=k % 4)
                g16_ = g16p.tile([128, Gmax * D], bf16, tag="g16")
                nc.vector.tensor_copy(out=g16_[:, :Gk * D], in_=gt_[:, :Gk * D])
                wt_ = wldp.tile([128, Gmax * 128], bf16, tag="w")
                nc.sync.dma_start(
                    out=wt_[:, :Gk * 128],
                    in_=wmat_t.ap()[:, gofs[k] * 128:(gofs[k] + Gk) * 128])
                sync_loads += 1
                if sync_loads == NHEAD and colsB:
                    nc.sync.dma_start(out=idxB[:],
                                      in_=idx16_t.ap()[:, colsA:TOTC])
                    idxB_loaded = True
                g = 0
                for T in range(NT):
                    for gg in range(GT[T][k]):
                        nc.tensor.matmul(
                            out=psts[T][:],
                            lhsT=wt_[:, g * 128:(g + 1) * 128],
                            rhs=g16_[:, g * D:(g + 1) * D],
                            start=(k == first_op[T] and gg == 0),
                            stop=(k == last_op[T] and gg == GT[T][k] - 1))
                        g += 1
                for T in range(NT):
                    if k == last_op[T]:
                        nc.vector.tensor_copy(out=dv16[T][:], in_=psts[T][:])

            if colsB and not idxB_loaded:
                nc.sync.dma_start(out=idxB[:], in_=idx16_t.ap()[:, colsA:TOTC])
            for T in range(NT):
                if first_op[T] < 0:
                    nc.vector.memset(dv16[T][:], 0.0)

            Ah_sb = constp.tile([128, NT * NJ * 128], bf16)
            nc.sync.dma_start(out=Ah_sb[:], in_=Ah_t.ap())
            At_sb = constp.tile([128, NT * NJ * 128], bf16)
            nc.sync.dma_start(out=At_sb[:], in_=At_t.ap())
            gh_sb = constp.tile([128, NJ], f32)
            nc.sync.dma_start(out=gh_sb[:], in_=gh_t.ap())
            gt_sb = constp.tile([128, NJ], f32)
            nc.sync.dma_start(out=gt_sb[:], in_=gt_t.ap())
            rone_sb = constp.tile([RELS, NJ * 128], f32)
            nc.sync.dma_start(out=rone_sb[:], in_=rone_t.ap())
            rel_sb = constp.tile([RELS, D], f32)
            nc.sync.dma_start(out=rel_sb[:], in_=rel_emb.ap())

            # ---- score phase ----
            score_sb = constp.tile([128, NJ], f32)
            for j in range(NJ):
                ph = spsump.tile([128, 128], f32, tag="ph")
                pt = spsump.tile([128, 128], f32, tag="pt")
                pr = spsump.tile([128, 128], f32, tag="pr")
                for T in range(NT):
                    nc.tensor.matmul(
                        out=ph[:],
                        lhsT=Ah_sb[:, (T * NJ + j) * 128:(T * NJ + j + 1) * 128],
                        rhs=dv16[T][:], start=(T == 0), stop=(T == NT - 1))
                for T in range(NT):
                    nc.tensor.matmul(
                        out=pt[:],
                        lhsT=At_sb[:, (T * NJ + j) * 128:(T * NJ + j + 1) * 128],
                        rhs=dv16[T][:], start=(T == 0), stop=(T == NT - 1))
                nc.tensor.matmul(
                    out=pr[:], lhsT=rone_sb[:, j * 128:(j + 1) * 128],
                    rhs=rel_sb[:], start=True, stop=True)

                t2 = workp.tile([128, D], f32, tag="t2")
                nc.vector.tensor_scalar(
                    out=t2[:], in0=old_h(j), scalar1=gh_sb[:, j:j + 1],
                    scalar2=None, op0=Alu.mult)
                hv = workp.tile([128, D], f32, tag="hv")
                nc.vector.tensor_tensor(out=hv[:], in0=ph[:], in1=t2[:],
                                        op=Alu.add)
                t4 = workp.tile([128, D], f32, tag="t4")
                nc.vector.tensor_scalar(
                    out=t4[:], in0=old_t(j), scalar1=gt_sb[:, j:j + 1],
                    scalar2=None, op0=Alu.mult)
                tv = workp.tile([128, D], f32, tag="tv")
                nc.vector.tensor_tensor(out=tv[:], in0=pt[:], in1=t4[:],
                                        op=Alu.add)
                p1 = workp.tile([128, D], f32, tag="p1")
                nc.vector.tensor_tensor(out=p1[:], in0=hv[:], in1=tv[:],
                                        op=Alu.mult)
                p2 = workp.tile([128, D], f32, tag="p2")
                nc.vector.tensor_tensor(out=p2[:], in0=p1[:], in1=pr[:],
                                        op=Alu.mult)
                nc.vector.reduce_sum(out=score_sb[:, j:j + 1], in_=p2[:],
                                     axis=mybir.AxisListType.X)
            nc.sync.dma_start(out=score_t.ap(), in_=score_sb[:])

    nc.compile()
    return nc


def _get_nc(static):
    key = ("v11", SP, MULTI_IND, static["NT"], tuple(map(tuple, static["GT"])),
           tuple(static["NI"]))
    if key not in _CACHE:
        _CACHE[key] = _build_nc(static)
    return _CACHE[key]


def kernel(head_index, rel_type, tail_index, node_emb, rel_emb,
           local_idx_map, sim_neighbors, sim_weights, degree_table):
    from concourse.bass_utils import run_bass_kernel_spmd

    head = np.asarray(head_index).astype(np.int64)
    rel = np.asarray(rel_type).astype(np.int64)
    tailv = np.asarray(tail_index).astype(np.int64)
    node_emb = np.ascontiguousarray(np.asarray(node_emb, dtype=np.float32))
    rel_emb = np.ascontiguousarray(np.asarray(rel_emb, dtype=np.float32))

    M, GCAP = M_DEF, GCAP_DEF
    while True:
        cores, extra = _prep_cores(head, rel, tailv, local_idx_map,
                                   sim_neighbors, sim_weights, degree_table,
                                   M, GCAP)
        if cores == "slots":
            M *= 2
        elif cores == "bucket":
            GCAP += 1
        else:
            break
    static, perm = extra

    nc = _get_nc(static)
    in_maps = []
    for c in range(NCORES):
        cc = cores[c]
        in_maps.append({
            "node_emb": node_emb, "rel_emb": rel_emb,
            "idx16": cc["idx16"], "wmat": cc["wmat"],
            "Ah": cc["Ah"], "At": cc["At"],
            "gh": cc["gh"], "gt": cc["gt"], "rone": cc["rone"],
            "headtail": cc["headtail"],
        })

    _CACHE["last_in_maps"] = in_maps
    res = run_bass_kernel_spmd(nc, in_maps, core_ids=list(range(NCORES)))
    _CACHE["last_result"] = res
    _CACHE["last_nc"] = nc
    _CACHE["last_perm"] = perm

    out = np.empty(B, np.float32)
    for c in range(NCORES):
        out[perm[c]] = res.results[c]["score"].T.reshape(-1)
    return out


# revision 18
# speedup vs baseline: 1.5191x; 1.0788x over previous
"""Trainium2 Bass kernel for nn_DistMultMod, v12.

Per core (BC=1024 triplets, balanced assignment):
  - comp slots (masked last-writer nodes, ~195/core): dv = sum_k w*node_emb[neigh]
    via dma_gather of neighbor rows bucketed by 32768-row block (16 merged ops,
    [T0 | T1] sections, T1 tail statically trimmed to max-over-cores), weights
    applied on DVE (per-group tensor_scalar, f32 w, bf16 out), then PE matmuls
    with BINARY one-hot W (fp8e4) accumulating into per-parity PSUM tiles.
  - old head/tail rows: 16 per-j indirect DMAs AFTER the comp gathers
    (DMA_INDIRECT descriptor-gen uses all 8 Q7 cores; running them last
    overlaps the comp drain tail instead of serializing before comps).
  - score: per j-tile: psum_h = sum_T Ah[T,j].T @ dv16[T]; pr = Rone.T @ rel;
    h = psum_h + gh*old_h; t = psum_t + gt*old_t; score = sum(h*t*pr).

v12 vs v11:
  - binary fp8 one-hot W (2.1MB -> 1.05MB HWDGE traffic) + exact f32 weights
    via DVE scaling fused with the bf16 cast.
  - gather indices sorted by address within each (block, T) section (HBM
    row locality for the latency-bound 512B random reads).
  - T1 sections trimmed to align16(max-over-cores n1): ~8% fewer random rows.
    gath buffers memset once at startup (unwritten tail rows must be finite).
  - inds after comps; single_packet=True; 64KB descriptor-ring carveout;
    explicit mlp library load first.
"""
import numpy as np
import ml_dtypes

BF16 = ml_dtypes.bfloat16
FP8 = ml_dtypes.float8_e4m3

B = 8192
NCORES = 8
BC = B // NCORES
D = 128
K = 64
N = 500000
RELS = 16
ND = 20000
NBLK = 16
BLK = 32768
NJ = BC // 128
LAM = 0.7
SP = True
M_DEF = 256
GCAP_DEF = 4

_CACHE = {}


def _assign_cores(head, rel, mask):
    last_of = {}
    for b in range(B):
        last_of[int(head[b])] = b
    is_writer = np.zeros(B, bool)
    for b in range(B):
        if mask[b] and last_of[int(head[b])] == b:
            is_writer[b] = True
    perm = [[] for _ in range(NCORES)]
    c = 0
    for b in np.flatnonzero(is_writer):
        perm[c % NCORES].append(int(b))
        c += 1
    c = 0
    for b in np.flatnonzero(~is_writer):
        while len(perm[c % NCORES]) >= BC:
            c += 1
        perm[c % NCORES].append(int(b))
        c += 1
    return np.array(perm, np.int64), last_of


def _prep_cores(head, rel, tailv, local_idx_map, sim_neighbors, sim_weights,
                degree_table, M, GCAP):
    NT = M // 128
    CAP = GCAP * 128
    mask = (rel >= 2) & (rel <= 4)
    local_idx_map = np.asarray(local_idx_map)
    sim_neighbors = np.asarray(sim_neighbors)
    sim_weights = np.asarray(sim_weights)
    degree_table = np.asarray(degree_table)

    perm, last_of = _assign_cores(head, rel, mask)

    raw = []
    for c in range(NCORES):
        pc = perm[c]
        slot_of = {}
        writers = []

        def slot_for(node):
            bw = last_of.get(int(node), -1)
            if bw < 0 or not mask[bw]:
                return M - 1
            s = slot_of.get(bw, -1)
            if s < 0:
                s = len(writers)
                slot_of[bw] = s
                writers.append(bw)
            return s

        slot_h = np.empty(BC, np.int32)
        slot_t = np.empty(BC, np.int32)
        for i in range(BC):
            slot_h[i] = slot_for(head[pc[i]])
            slot_t[i] = slot_for(tailv[pc[i]])
        m = len(writers)
        if m > M - 1:
            return "slots", None

        a = np.zeros(M, np.float32)
        wl = np.array(writers, dtype=np.int64)
        if m:
            ls = local_idx_map[head[wl]]
            neigh_rows = sim_neighbors[ls].astype(np.int64)
            w_rows = sim_weights[ls].astype(np.float32)
            deg = degree_table[ls, rel[wl] - 2].astype(np.float32)
            a[:m] = LAM * np.exp(-LAM * deg) + 0.2
        else:
            neigh_rows = np.zeros((0, K), np.int64)
            w_rows = np.zeros((0, K), np.float32)

        srows = np.repeat(np.arange(m), K)
        rnodes = neigh_rows.reshape(-1)
        wvals = w_rows.reshape(-1)
        Tpar = (srows % NT).astype(np.int64)
        blk = rnodes >> 15
        pcol = (srows // NT).astype(np.int64)
        off = (rnodes & (BLK - 1)).astype(np.int64)
        order = np.lexsort((off, Tpar, blk))   # addr-sorted within section
        raw.append(dict(slot_h=slot_h, slot_t=slot_t, a=a, pc=pc,
                        sTs=Tpar[order], sblk=blk[order], soff=off[order],
                        sw=wvals[order], sp=pcol[order]))

    cnt = np.zeros((NCORES, NBLK, NT), np.int64)
    for c in range(NCORES):
        r = raw[c]
        np.add.at(cnt[c], (r["sblk"], r["sTs"]), 1)
    if cnt.max() > CAP:
        return "bucket", None

    mx = cnt.max(axis=0)                      # [NBLK, NT]
    # section sizes: leading Ts padded to 128-multiples (pure groups),
    # final T tail trimmed to align16(max count over cores)
    S = np.zeros((NBLK, NT), np.int64)
    for op in range(NBLK):
        for T in range(NT):
            if T < NT - 1:
                S[op, T] = int(np.ceil(mx[op, T] / 128)) * 128
            else:
                S[op, T] = ((int(mx[op, T]) + 15) // 16) * 16
    GT = [[int(np.ceil(S[op, T] / 128)) for op in range(NBLK)]
          for T in range(NT)]
    NI = [int(S[op].sum()) for op in range(NBLK)]
    Gmax = max(sum(int(np.ceil(S[op, T] / 128)) for T in range(NT))
               for op in range(NBLK)) or 1
    static = dict(NT=NT, GT=GT, NI=NI, M=M, Gmax=Gmax)

    cores = []
    for c in range(NCORES):
        r = raw[c]
        a, slot_h, slot_t, pc = r["a"], r["slot_h"], r["slot_t"], r["pc"]
        idx_cols = []
        wone = []        # binary one-hot groups
        wcols = []       # per-group per-row weight columns [128]
        for op in range(NBLK):
            gbase = 0
            idx_op = np.zeros(NI[op], np.int16)
            for T in range(NT):
                sel = (r["sblk"] == op) & (r["sTs"] == T)
                offs = r["soff"][sel]
                ws = r["sw"][sel]
                ps = r["sp"][sel]
                n = len(offs)
                sec = int(S[op, T])
                assert n <= sec
                s0 = int(S[op, :T].sum())
                idx_op[s0:s0 + n] = offs
                ng = int(np.ceil(sec / 128))
                Wb = np.zeros((ng, 128, 128), np.float32)
                Wc = np.zeros((ng * 128,), np.float32)
                pos = np.arange(n)
                Wb[pos // 128, pos % 128, ps] = 1.0
                Wc[:n] = ws
                wone.append(Wb.reshape(ng * 128, 128))
                wcols.append(Wc.reshape(ng, 128))
                gbase += ng
            idx_cols.append(idx_op)
        idx_all = np.concatenate(idx_cols)
        tc = len(idx_all) // 16
        idx16 = np.tile(idx_all.reshape(tc, 16).T, (8, 1))
        wmat = np.concatenate(wone, axis=0)
        wmat = np.ascontiguousarray(
            wmat.reshape(-1, 128, 128).transpose(1, 0, 2)
            .reshape(128, -1)).astype(FP8)
        wcol = np.ascontiguousarray(
            np.concatenate(wcols, axis=0).T)        # [128, TOTG]

        i = np.arange(BC)
        j_, t_ = i // 128, i % 128
        Ah = np.zeros((128, NT * NJ * 128), np.float32)
        T_, p_ = slot_h % NT, slot_h // NT
        Ah[p_, (T_ * NJ + j_) * 128 + t_] = a[slot_h]
        At = np.zeros((128, NT * NJ * 128), np.float32)
        T_, p_ = slot_t % NT, slot_t // NT
        At[p_, (T_ * NJ + j_) * 128 + t_] = a[slot_t]
        gh = np.ascontiguousarray((1.0 - a[slot_h]).reshape(NJ, 128).T)
        gt = np.ascontiguousarray((1.0 - a[slot_t]).reshape(NJ, 128).T)
        rone = np.zeros((RELS, NJ * 128), np.float32)
        rone[rel[pc], i] = 1.0

        hmat = head[pc].reshape(NJ, 128).T
        tmat = tailv[pc].reshape(NJ, 128).T
        headtail = np.concatenate([hmat, tmat], axis=1)

        cores.append(dict(
            idx16=np.ascontiguousarray(idx16),
            wmat=wmat,
            wcol=wcol.astype(np.float32),
            Ah=np.ascontiguousarray(Ah).astype(BF16),
            At=np.ascontiguousarray(At).astype(BF16),
            gh=gh.astype(np.float32),
            gt=gt.astype(np.float32),
            rone=np.ascontiguousarray(rone),
            headtail=np.ascontiguousarray(headtail).astype(np.int32),
        ))
    return cores, (static, perm)


def _build_nc(static):
    import concourse.bass as bass
    import concourse.bacc as bacc
    import concourse.mybir as mybir
    import concourse.tile as tile
    from concourse import library_config

    NT = static["NT"]
    GT = static["GT"]
    NI = static["NI"]
    Gmax = static["Gmax"]
    NOP = len(NI)
    f32 = mybir.dt.float32
    bf16 = mybir.dt.bfloat16
    fp8 = mybir.dt.float8e4
    i32 = mybir.dt.int32
    i16 = mybir.dt.int16
    Alu = mybir.AluOpType

    TOTC = sum(ni // 16 for ni in NI)
    colo = np.cumsum([0] + [ni // 16 for ni in NI]).tolist()
    gofs = []
    acc = 0
    for op in range(NOP):
        gofs.append(acc)
        acc += sum(GT[T][op] for T in range(NT))
    TOTG = acc

    nc = bacc.Bacc("TRN2", target_bir_lowering=False, debug=False,
                   num_devices=NCORES, num_swdge_queues=4,
                   dynamic_dma_scratch_size=65536)

    node_emb = nc.dram_tensor("node_emb", [N, D], f32, kind="ExternalInput")
    rel_emb = nc.dram_tensor("rel_emb", [RELS, D], f32, kind="ExternalInput")
    idx16_t = nc.dram_tensor("idx16", [128, TOTC], i16, kind="ExternalInput")
    wmat_t = nc.dram_tensor("wmat", [128, TOTG * 128], fp8,
                            kind="ExternalInput")
    wcol_t = nc.dram_tensor("wcol", [128, TOTG], f32, kind="ExternalInput")
    Ah_t = nc.dram_tensor("Ah", [128, NT * NJ * 128], bf16, kind="ExternalInput")
    At_t = nc.dram_tensor("At", [128, NT * NJ * 128], bf16, kind="ExternalInput")
    gh_t = nc.dram_tensor("gh", [128, NJ], f32, kind="ExternalInput")
    gt_t = nc.dram_tensor("gt", [128, NJ], f32, kind="ExternalInput")
    rone_t = nc.dram_tensor("rone", [RELS, NJ * 128], f32, kind="ExternalInput")
    ht_t = nc.dram_tensor("headtail", [128, 2 * NJ], i32, kind="ExternalInput")
    score_t = nc.dram_tensor("score", [128, NJ], f32, kind="ExternalOutput")

    first_op = [min((op for op in range(NOP) if GT[T][op]), default=-1)
                for T in range(NT)]
    last_op = [max((op for op in range(NOP) if GT[T][op]), default=-1)
               for T in range(NT)]

    NHEAD = min(4, NOP)
    colsA = colo[NHEAD]
    colsB = TOTC - colsA

    with tile.TileContext(nc) as tc:
        with tc.tile_pool(name="const", bufs=1) as constp, \
             tc.tile_pool(name="old", bufs=1) as oldp, \
             tc.tile_pool(name="gath", bufs=8) as gathp, \
             tc.tile_pool(name="g16", bufs=8) as g16p, \
             tc.tile_pool(name="wld", bufs=8) as wldp, \
             tc.tile_pool(name="cpsum", bufs=1, space="PSUM") as cpsump, \
             tc.tile_pool(name="spsum", bufs=2, space="PSUM") as spsump, \
             tc.tile_pool(name="work", bufs=4) as workp:

            nc.gpsimd.load_library(library_config.mlp)

            ht_sb = constp.tile([128, 2 * NJ], i32)
            nc.sync.dma_start(out=ht_sb[:], in_=ht_t.ap())
            idxA = constp.tile([128, colsA], i16)
            nc.sync.dma_start(out=idxA[:], in_=idx16_t.ap()[:, :colsA])
            wcol_sb = constp.tile([128, TOTG], f32)
            nc.sync.dma_start(out=wcol_sb[:], in_=wcol_t.ap())
            if colsB:
                idxB = constp.tile([128, colsB], i16)

            psts = [cpsump.tile([128, 128], f32, tag=f"ps{T}", name=f"ps{T}")
                    for T in range(NT)]
            dv16 = [constp.tile([128, D], bf16, tag=f"dv{T}", name=f"dv{T}")
                    for T in range(NT)]
            oh_tiles = [oldp.tile([128, D], f32, tag=f"oh{j}", name=f"oh{j}")
                        for j in range(NJ)]
            ot_tiles = [oldp.tile([128, D], f32, tag=f"ot{j}", name=f"ot{j}")
                        for j in range(NJ)]

            # memset gather buffers: trimmed tails leave unwritten rows that
            # feed matmuls (x W=0); they must be finite, not NaN garbage.
            gtiles = []
            for _ in range(8):
                z = gathp.tile([128, Gmax * D], f32, tag="g")
                nc.vector.memset(z[:], 0.0)
                gtiles.append(z)

            # ---- comp gathers: contiguous, queues 0-3 ----
            sync_loads = 0
            idxB_loaded = False
            for k in range(NOP):
                Gk = sum(GT[T][k] for T in range(NT))
                if Gk == 0:
                    continue
                ni = NI[k]
                c0, c1 = colo[k], colo[k + 1]
                if c1 <= colsA:
                    isrc = idxA[:, c0:c1]
                else:
                    isrc = idxB[:, c0 - colsA:c1 - colsA]
                gt_ = gathp.tile([128, Gmax * D], f32, tag="g")
                nc.gpsimd.dma_gather(
                    out_ap=gt_[:, :Gk * D].rearrange("p (b d) -> p b d", d=D),
                    in_ap=node_emb.ap()[k * BLK:min((k + 1) * BLK, N), :],
                    idxs_ap=isrc,
                    num_idxs=ni, num_idxs_reg=ni, elem_size=D,
                    single_packet=SP, queue_num=k % 4)
                g16_ = g16p.tile([128, Gmax * D], bf16, tag="g16")
                wt_ = wldp.tile([128, Gmax * 128], fp8, tag="w")
                nc.sync.dma_start(
                    out=wt_[:, :Gk * 128],
                    in_=wmat_t.ap()[:, gofs[k] * 128:(gofs[k] + Gk) * 128])
                sync_loads += 1
                if sync_loads == NHEAD and colsB:
                    nc.sync.dma_start(out=idxB[:],
                                      in_=idx16_t.ap()[:, colsA:TOTC])
                    idxB_loaded = True
                # weight-scale + cast per group (w is per-row = per-partition;
                # sections are group-aligned so group g = rows [g*128,(g+1)*128))
                for g in range(Gk):
                    nc.vector.tensor_scalar(
                        out=g16_[:, g * D:(g + 1) * D],
                        in0=gt_[:, g * D:(g + 1) * D],
                        scalar1=wcol_sb[:, gofs[k] + g:gofs[k] + g + 1],
                        scalar2=None, op0=Alu.mult)
                g = 0
                for T in range(NT):
                    for gg in range(GT[T][k]):
                        nc.tensor.matmul(
                            out=psts[T][:],
                            lhsT=wt_[:, g * 128:(g + 1) * 128],
                            rhs=g16_[:, g * D:(g + 1) * D],
                            start=(k == first_op[T] and gg == 0),
                            stop=(k == last_op[T] and gg == GT[T][k] - 1))
                        g += 1
                for T in range(NT):
                    if k == last_op[T]:
                        nc.vector.tensor_copy(out=dv16[T][:], in_=psts[T][:])

            if colsB and not idxB_loaded:
                nc.sync.dma_start(out=idxB[:], in_=idx16_t.ap()[:, colsA:TOTC])
            for T in range(NT):
                if first_op[T] < 0:
                    nc.vector.memset(dv16[T][:], 0.0)

            # ---- old head/tail rows (after comps: overlap comp drain) ----
            for j in range(NJ):
                nc.gpsimd.indirect_dma_start(
                    out=oh_tiles[j][:], out_offset=None, in_=node_emb.ap(),
                    in_offset=bass.IndirectOffsetOnAxis(
                        ap=ht_sb[:, j:j + 1], axis=0))
                nc.gpsimd.indirect_dma_start(
                    out=ot_tiles[j][:], out_offset=None, in_=node_emb.ap(),
                    in_offset=bass.IndirectOffsetOnAxis(
                        ap=ht_sb[:, NJ + j:NJ + j + 1], axis=0))

            Ah_sb = constp.tile([128, NT * NJ * 128], bf16)
            nc.sync.dma_start(out=Ah_sb[:], in_=Ah_t.ap())
            At_sb = constp.tile([128, NT * NJ * 128], bf16)
            nc.sync.dma_start(out=At_sb[:], in_=At_t.ap())
            gh_sb = constp.tile([128, NJ], f32)
            nc.sync.dma_start(out=gh_sb[:], in_=gh_t.ap())
            gt_sb = constp.tile([128, NJ], f32)
            nc.sync.dma_start(out=gt_sb[:], in_=gt_t.ap())
            rone_sb = constp.tile([RELS, NJ * 128], f32)
            nc.sync.dma_start(out=rone_sb[:], in_=rone_t.ap())
            rel_sb = constp.tile([RELS, D], f32)
            nc.sync.dma_start(out=rel_sb[:], in_=rel_emb.ap())

            # ---- score phase ----
            score_sb = constp.tile([128, NJ], f32)
            for j in range(NJ):
                ph = spsump.tile([128, 128], f32, tag="ph")
                pt = spsump.tile([128, 128], f32, tag="pt")
                pr = spsump.tile([128, 128], f32, tag="pr")
                for T in range(NT):
                    nc.tensor.matmul(
                        out=ph[:],
                        lhsT=Ah_sb[:, (T * NJ + j) * 128:(T * NJ + j + 1) * 128],
                        rhs=dv16[T][:], start=(T == 0), stop=(T == NT - 1))
                for T in range(NT):
                    nc.tensor.matmul(
                        out=pt[:],
                        lhsT=At_sb[:, (T * NJ + j) * 128:(T * NJ + j + 1) * 128],
                        rhs=dv16[T][:], start=(T == 0), stop=(T == NT - 1))
                nc.tensor.matmul(
                    out=pr[:], lhsT=rone_sb[:, j * 128:(j + 1) * 128],
                    rhs=rel_sb[:], start=True, stop=True)

                t2 = workp.tile([128, D], f32, tag="t2")
                nc.vector.tensor_scalar(
                    out=t2[:], in0=oh_tiles[j][:], scalar1=gh_sb[:, j:j + 1],
                    scalar2=None, op0=Alu.mult)
                hv = workp.tile([128, D], f32, tag="hv")
                nc.vector.tensor_tensor(out=hv[:], in0=ph[:], in1=t2[:],
                                        op=Alu.add)
                t4 = workp.tile([128, D], f32, tag="t4")
                nc.vector.tensor_scalar(
                    out=t4[:], in0=ot_tiles[j][:], scalar1=gt_sb[:, j:j + 1],
                    scalar2=None, op0=Alu.mult)
                tv = workp.tile([128, D], f32, tag="tv")
                nc.vector.tensor_tensor(out=tv[:], in0=pt[:], in1=t4[:],
                                        op=Alu.add)
                p1 = workp.tile([128, D], f32, tag="p1")
                nc.vector.tensor_tensor(out=p1[:], in0=hv[:], in1=tv[:],
                                        op=Alu.mult)
                p2 = workp.tile([128, D], f32, tag="p2")
                nc.vector.tensor_tensor(out=p2[:], in0=p1[:], in1=pr[:],
                                        op=Alu.mult)
                nc.vector.reduce_sum(out=score_sb[:, j:j + 1], in_=p2[:],
                                     axis=mybir.AxisListType.X)
            nc.sync.dma_start(out=score_t.ap(), in_=score_sb[:])

    nc.compile()
    return nc


def _get_nc(static):
    key = ("v12", SP, static["NT"], tuple(map(tuple, static["GT"])),
           tuple(static["NI"]))
    if key not in _CACHE:
        _CACHE[key] = _build_nc(static)
    return _CACHE[key]


def kernel(head_index, rel_type, tail_index, node_emb, rel_emb,
           local_idx_map, sim_neighbors, sim_weights, degree_table):
    from concourse.bass_utils import run_bass_kernel_spmd

    head = np.asarray(head_index).astype(np.int64)
    rel = np.asarray(rel_type).astype(np.int64)
    tailv = np.asarray(tail_index).astype(np.int64)
    node_emb = np.ascontiguousarray(np.asarray(node_emb, dtype=np.float32))
    rel_emb = np.ascontiguousarray(np.asarray(rel_emb, dtype=np.float32))

    M, GCAP = M_DEF, GCAP_DEF
    while True:
        cores, extra = _prep_cores(head, rel, tailv, local_idx_map,
                                   sim_neighbors, sim_weights, degree_table,
                                   M, GCAP)
        if cores == "slots":
            M *= 2
        elif cores == "bucket":
            GCAP += 1
        else:
            break
    static, perm = extra

    nc = _get_nc(static)
    in_maps = []
    for c in range(NCORES):
        cc = cores[c]
        in_maps.append({
            "node_emb": node_emb, "rel_emb": rel_emb,
            "idx16": cc["idx16"], "wmat": cc["wmat"], "wcol": cc["wcol"],
            "Ah": cc["Ah"], "At": cc["At"],
            "gh": cc["gh"], "gt": cc["gt"], "rone": cc["rone"],
            "headtail": cc["headtail"],
        })

    _CACHE["last_in_maps"] = in_maps
    res = run_bass_kernel_spmd(nc, in_maps, core_ids=list(range(NCORES)))
    _CACHE["last_result"] = res
    _CACHE["last_nc"] = nc
    _CACHE["last_perm"] = perm

    out = np.empty(B, np.float32)
    for c in range(NCORES):
        out[perm[c]] = res.results[c]["score"].T.reshape(-1)
    return out
